# revision 7
# baseline (speedup 1.0000x reference)
"""EpisodicSlotReader Trainium2 kernel.

Math per batch row b (B=2048, S=256, D=512):
    qn    = q / (||q|| + 1e-6)
    kn_s  = k_s / (||k_s|| + 1e-6)
    sim_s = kn_s . qn                       = (k_s . q) / ((||k_s||+eps)(||q||+eps))
    logit_s = sim_s + 0.5*ln(max(str_s, 1e-3)) - 0.02*age_s - 1000*[str_s <= 1e-3]
    w     = softmax(logit)
    read  = RMSNorm(sum_s w_s * v_s) * scale
Outputs: (read [B,D], w [B,S], logit [B,S]).

Strategy: pure data parallel over B across 8 cores (256 rows/core).
Layout: batch rows on the 128 SBUF partitions, D/S in the free dim, so every
reduction is a free-dim reduction and each partition owns its own q row.
Memory-bound: 2 GiB of key/value reads dominate; keys are streamed once for
the logit pass and values once for the weighted-sum pass.
"""

import numpy as np

B, S, D = 2048, 256, 512
N_CORES = 8
B_SH = B // N_CORES          # rows per core
P = 128                      # SBUF partitions
N_BLK = B_SH // P            # row blocks per core
NS = 8                       # slots per streamed DMA chunk (2 MiB per load)

EPS = 1e-6
STR_CLIP = 0.001
STR_BOOST = 0.5
AGE_PEN = 0.02
MASK_PEN = -1000.0
RMS_EPS = 1e-06

_CACHE = {}


def build_program(reps=1):
    import contextlib

    import concourse.bacc as bacc
    import concourse.bass as bass
    import concourse.tile as tile
    from concourse import mybir

    f32 = mybir.dt.float32
    OP = mybir.AluOpType
    AF = mybir.ActivationFunctionType

    nc = bacc.Bacc("TRN2", target_bir_lowering=False, debug=False)

    q_h = nc.dram_tensor("q", [B_SH, D], f32, kind="ExternalInput")
    keys_h = nc.dram_tensor("keys", [B_SH, S, D], f32, kind="ExternalInput")
    vals_h = nc.dram_tensor("vals", [B_SH, S, D], f32, kind="ExternalInput")
    age_h = nc.dram_tensor("age", [B_SH, S], f32, kind="ExternalInput")
    str_h = nc.dram_tensor("strength", [B_SH, S], f32, kind="ExternalInput")
    scale_h = nc.dram_tensor("scale", [D], f32, kind="ExternalInput")
    oread_h = nc.dram_tensor("out_read", [B_SH, D], f32, kind="ExternalOutput")
    ow_h = nc.dram_tensor("out_w", [B_SH, S], f32, kind="ExternalOutput")
    olog_h = nc.dram_tensor("out_logits", [B_SH, S], f32, kind="ExternalOutput")

    with tile.TileContext(nc) as tc:
        with (
            tc.tile_pool(name="singles", bufs=1) as singles,
            tc.tile_pool(name="kv", bufs=3) as kv,
            tc.tile_pool(name="blk", bufs=2) as blk,
            tc.tile_pool(name="row", bufs=2) as row,
            tc.tile_pool(name="small", bufs=2) as small,
        ):
            # scale broadcast to all partitions, loaded once
            eps_t = singles.tile([P, 1], f32)
            nc.vector.memset(eps_t, RMS_EPS)
            scale_t = singles.tile([P, D], f32)
            scale_ap = scale_h[:]
            nc.sync.dma_start(
                out=scale_t,
                in_=bass.AP(
                    tensor=scale_ap.tensor,
                    offset=scale_ap.offset,
                    ap=[[0, P]] + list(scale_ap.ap),
                ),
            )

            rep_ctx = (
                tc.For_i(0, reps, 1) if reps > 1 else contextlib.nullcontext()
            )
            with rep_ctx:
              for ib in range(N_BLK):
                b0 = ib * P
                # ---- query block + 1/(||q||+eps) ----
                q_t = blk.tile([P, D], f32, tag="q")
                nc.sync.dma_start(out=q_t, in_=q_h[b0 : b0 + P, :])
                sq_act = blk.tile([P, D], f32, tag="sq_act")  # ACT scratch
                ssq = small.tile([P, 1], f32, tag="ssq")
                nc.scalar.activation(out=sq_act, in_=q_t, func=AF.Square, accum_out=ssq)
                qfac = small.tile([P, 1], f32, tag="qfac")
                nc.scalar.activation(out=qfac, in_=ssq, func=AF.Sqrt)
                nc.vector.tensor_scalar_add(out=qfac, in0=qfac, scalar1=EPS)
                nc.vector.reciprocal(out=qfac, in_=qfac)

                # ---- strength/age logit bias ----
                st_t = row.tile([P, S], f32, tag="st")
                nc.sync.dma_start(out=st_t, in_=str_h[b0 : b0 + P, :])
                age_t = row.tile([P, S], f32, tag="age")
                nc.sync.dma_start(out=age_t, in_=age_h[b0 : b0 + P, :])
                clip_t = row.tile([P, S], f32, tag="clip")
                nc.vector.tensor_scalar_max(out=clip_t, in0=st_t, scalar1=STR_CLIP)
                lg_t = row.tile([P, S], f32, tag="lg")
                nc.scalar.activation(out=lg_t, in_=clip_t, func=AF.Ln)
                pen_t = row.tile([P, S], f32, tag="pen")
                nc.vector.tensor_scalar(
                    out=pen_t, in0=st_t, scalar1=STR_CLIP, scalar2=MASK_PEN,
                    op0=OP.is_le, op1=OP.mult,
                )
                lb_t = row.tile([P, S], f32, tag="lb")
                nc.vector.scalar_tensor_tensor(
                    out=lb_t, in0=lg_t, scalar=STR_BOOST, in1=pen_t,
                    op0=OP.mult, op1=OP.add,
                )
                nc.vector.scalar_tensor_tensor(
                    out=lb_t, in0=age_t, scalar=-AGE_PEN, in1=lb_t,
                    op0=OP.mult, op1=OP.add,
                )

                # ---- pass 1: stream keys; dot(k,q) and sum(k^2) per slot ----
                dots_t = row.tile([P, S], f32, tag="dots")
                ssk_t = row.tile([P, S], f32, tag="ssk")
                dve_scr = blk.tile([P, D], f32, tag="dve_scr")  # DVE scratch out
                for ic in range(S // NS):
                    k_t = kv.tile([P, NS, D], f32, tag="k")
                    nc.sync.dma_start(
                        out=k_t, in_=keys_h[b0 : b0 + P, ic * NS : (ic + 1) * NS, :]
                    )
                    for j in range(NS):
                        s = ic * NS + j
                        # accum_out gets qfac * (k_s . q); elementwise out is scratch
                        nc.vector.scalar_tensor_tensor(
                            out=dve_scr, in0=k_t[:, j, :], scalar=qfac, in1=q_t,
                            op0=OP.mult, op1=OP.mult,
                            accum_out=dots_t[:, s : s + 1],
                        )
                        nc.scalar.activation(
                            out=sq_act, in_=k_t[:, j, :], func=AF.Square,
                            accum_out=ssk_t[:, s : s + 1],
                        )

                # ---- logits ----
                rk_t = row.tile([P, S], f32, tag="rk")
                nc.scalar.activation(out=rk_t, in_=ssk_t, func=AF.Sqrt)
                nc.vector.tensor_scalar_add(out=rk_t, in0=rk_t, scalar1=EPS)
                nc.vector.reciprocal(out=rk_t, in_=rk_t)
                sim_t = row.tile([P, S], f32, tag="sim")
                nc.vector.tensor_tensor(
                    out=sim_t, in0=dots_t, in1=rk_t, op=OP.mult
                )
                log_t = row.tile([P, S], f32, tag="logit")
                nc.vector.tensor_tensor(
                    out=log_t, in0=sim_t, in1=lb_t, op=OP.add
                )
                nc.sync.dma_start(out=olog_h[b0 : b0 + P, :], in_=log_t)

                # ---- softmax over S (free dim) ----
                negmax = small.tile([P, 1], f32, tag="negmax")
                nc.vector.tensor_reduce(
                    out=negmax, in_=log_t, axis=mybir.AxisListType.X,
                    op=OP.max, negate=True,
                )
                e_t = row.tile([P, S], f32, tag="e")
                sumexp = small.tile([P, 1], f32, tag="sumexp")
                nc.scalar.activation(
                    out=e_t, in_=log_t, func=AF.Exp, bias=negmax, scale=1.0,
                    accum_out=sumexp,
                )
                winv = small.tile([P, 1], f32, tag="winv")
                nc.vector.reciprocal(out=winv, in_=sumexp)
                w_t = row.tile([P, S], f32, tag="w")
                nc.vector.tensor_scalar_mul(out=w_t, in0=e_t, scalar1=winv)
                nc.sync.dma_start(out=ow_h[b0 : b0 + P, :], in_=w_t)

                # ---- pass 2: stream values; read = sum_s w_s * v_s ----
                acc_t = blk.tile([P, D], f32, tag="acc")
                nc.vector.memset(acc_t, 0.0)
                for ic in range(S // NS):
                    v_t = kv.tile([P, NS, D], f32, tag="v")
                    nc.sync.dma_start(
                        out=v_t, in_=vals_h[b0 : b0 + P, ic * NS : (ic + 1) * NS, :]
                    )
                    for j in range(NS):
                        s = ic * NS + j
                        nc.vector.scalar_tensor_tensor(
                            out=acc_t, in0=v_t[:, j, :], scalar=w_t[:, s : s + 1],
                            in1=acc_t, op0=OP.mult, op1=OP.add,
                        )

                # ---- RMSNorm * scale ----
                ssr = small.tile([P, 1], f32, tag="ssr")
                nc.scalar.activation(
                    out=sq_act, in_=acc_t, func=AF.Square, accum_out=ssr
                )
                rms = small.tile([P, 1], f32, tag="rms")
                nc.scalar.activation(
                    out=rms, in_=ssr, func=AF.Sqrt, bias=eps_t, scale=1.0 / D
                )
                rinv = small.tile([P, 1], f32, tag="rinv")
                nc.vector.reciprocal(out=rinv, in_=rms)
                out_t = blk.tile([P, D], f32, tag="out")
                nc.vector.scalar_tensor_tensor(
                    out=out_t, in0=acc_t, scalar=rinv, in1=scale_t,
                    op0=OP.mult, op1=OP.mult,
                )
                nc.sync.dma_start(out=oread_h[b0 : b0 + P, :], in_=out_t)

    nc.compile()
    return nc


def get_nc():
    if "nc" not in _CACHE:
        _CACHE["nc"] = build_program()
    return _CACHE["nc"]


def kernel(q_win, epi_keys, epi_vals, epi_age, epi_strength, scale):
    from concourse.bass_utils import run_bass_kernel_spmd

    nc = get_nc()
    q_win = np.ascontiguousarray(q_win, dtype=np.float32)
    epi_keys = np.ascontiguousarray(epi_keys, dtype=np.float32)
    epi_vals = np.ascontiguousarray(epi_vals, dtype=np.float32)
    epi_age = np.ascontiguousarray(epi_age, dtype=np.float32)
    epi_strength = np.ascontiguousarray(epi_strength, dtype=np.float32)
    scale = np.ascontiguousarray(scale, dtype=np.float32)

    in_maps = []
    for c in range(N_CORES):
        r = slice(c * B_SH, (c + 1) * B_SH)
        in_maps.append(
            {
                "q": q_win[r],
                "keys": epi_keys[r],
                "vals": epi_vals[r],
                "age": epi_age[r],
                "strength": epi_strength[r],
                "scale": scale,
            }
        )
    res = run_bass_kernel_spmd(nc, in_maps, list(range(N_CORES)))
    read = np.concatenate([res.results[c]["out_read"] for c in range(N_CORES)], axis=0)
    w = np.concatenate([res.results[c]["out_w"] for c in range(N_CORES)], axis=0)
    logits = np.concatenate(
        [res.results[c]["out_logits"] for c in range(N_CORES)], axis=0
    )
    return read.astype(np.float32), w.astype(np.float32), logits.astype(np.float32)
